# revision 9
# baseline (speedup 1.0000x reference)
"""Trainium2 Bass kernel: batched int8 dequant-BMM.

out[b] = (x[b].f32 - a_zp) @ (y[b].f32 - b_zp) * alpha
  x: [96, 1024, 64] int8, y: [96, 64, 1024] int8 -> out: [96, 1024, 1024] f32

Sharding: batch dim 96 -> 12 per core across 8 cores (pure data parallel).

Per-core pipeline (batch pair at a time; even batch on partitions 0-63,
odd batch on partitions 64-127 so the K=64 contractions row-tile the PE):
  DMA x,y int8 -> ACT dequant to bf16 (exact: integers < 256)
  -> PE transpose x tiles (bf16 identity matmul) -> DVE copy to SBUF
  -> PE matmul bf16 x bf16 -> fp32 PSUM (exact: sums < 2^24)
  -> ACT/DVE copy PSUM->SBUF fused with *alpha, cast to fp16
  -> DMA out fp16 -> host upcasts to fp32.

The PSUM value is the exact integer result; alpha-scale + fp16 round
gives max rel err 2^-11 ~= 4.9e-4 (all nonzero outputs are fp16
normals), 40x inside the 2e-2 gate, and halves the store traffic that
bounds this kernel (memory regime: 4.2 MB output per batch).
"""

import numpy as np

B, S, D = 96, 1024, 64
N_CORES = 8
BPC = B // N_CORES  # batches per core = 12
NPAIRS = BPC // 2

_cache = {}


def _build(az: float, bz: float, al: float):
    key = (az, bz, al)
    if key in _cache:
        return _cache[key]

    from contextlib import ExitStack

    import concourse.mybir as mybir
    import concourse.tile as tile
    from concourse import bacc

    f32 = mybir.dt.float32
    f16 = mybir.dt.float16
    bf16 = mybir.dt.bfloat16
    i8 = mybir.dt.int8
    AF = mybir.ActivationFunctionType

    nc = bacc.Bacc(
        "TRN2", target_bir_lowering=False, debug=False, num_devices=N_CORES
    )
    x_d = nc.dram_tensor("x", [BPC, S, D], i8, kind="ExternalInput").ap()
    y_d = nc.dram_tensor("y", [BPC, D, S], i8, kind="ExternalInput").ap()
    o_d = nc.dram_tensor("out", [BPC, S, S], f16, kind="ExternalOutput").ap()

    # Row-residue tiling: m-tile r (r in 0..7) covers rows {8p + r}.
    # This makes the x load contiguous per partition (512B runs) and the
    # store rows of one partition contiguous in DRAM (gsize*4KB runs).
    # x[2c+bt, 8p+r, d] -> xv[p, c, bt, r, d]
    xv = x_d.rearrange("(c b2) (p r) d -> p c b2 r d", b2=2, p=128)
    # y[2c+bt, d, s] -> yv[bt*64+d, c, s]  (contiguous in DRAM)
    yv = y_d.rearrange("(c b2) d s -> (b2 d) c s", b2=2)
    # out[b, 8p+r, t] <- ovn[b, p, r, t]
    ovn = o_d.rearrange("b (p r) t -> b p r t", p=128, r=8)

    with tile.TileContext(nc) as tc, ExitStack() as ctx:
        const_pool = ctx.enter_context(tc.tile_pool(name="const", bufs=1))
        # all 6 x-pair tiles live at once: loads are issued up front
        xin_pool = ctx.enter_context(tc.tile_pool(name="xin", bufs=NPAIRS))
        yin_pool = ctx.enter_context(tc.tile_pool(name="yin", bufs=1))
        xbf_pool = ctx.enter_context(tc.tile_pool(name="xbf", bufs=2))
        ybf_pool = ctx.enter_context(tc.tile_pool(name="ybf", bufs=3))
        xt_pool = ctx.enter_context(tc.tile_pool(name="xt", bufs=3))
        stage_pool = ctx.enter_context(tc.tile_pool(name="stage", bufs=9))
        tpsum_pool = ctx.enter_context(
            tc.tile_pool(name="tpsum", bufs=2, space="PSUM")
        )
        mpsum_pool = ctx.enter_context(
            tc.tile_pool(name="mpsum", bufs=3, space="PSUM")
        )

        # Identity as a baked constant (avoids serializing gpsimd early).
        import ml_dtypes

        ident_dram = nc.inline_tensor(
            np.eye(128, dtype=ml_dtypes.bfloat16), name="ident128"
        ).ap()
        ident = const_pool.tile([128, 128], bf16)
        nc.sync.dma_start(out=ident[:], in_=ident_dram)

        # HAM warmup: ~3.4us of dummy matmuls while PE is otherwise idle
        # (waiting on loads) flips the PE clock gate from 1.2 to 2.4 GHz
        # before the real matmul stream starts at ~7.5us. Result is never
        # read. Data arrives via sync DMA (~3.5us) — engine memsets can't
        # run before ~6us (preamble), which is too late to matter.
        warm_dram = nc.inline_tensor(
            np.ones((128, 512), dtype=ml_dtypes.bfloat16), name="warm512"
        ).ap()
        warm_sb = const_pool.tile([128, 512], bf16)
        nc.sync.dma_start(out=warm_sb[:], in_=warm_dram)
        warm_ps = mpsum_pool.tile([128, S], f32, tag="mpsum")
        for w in range(8):
            nh = w % 2
            nc.tensor.matmul(
                warm_ps[:, nh * 512 : (nh + 1) * 512],
                warm_sb[:, :128],
                warm_sb[:],
                start=True,
                stop=True,
            )

        # All loads ride HWDGE (no SWDGE at all: SDMA engines 7/15 are
        # documented slower under SWDGE descriptor-ring port traffic and
        # intermittently straggle the store stream by ~17us). Everything
        # loads up front into the otherwise-idle 0-10us DMA window:
        # pairs 0-2 on the sync ring (free until the first store at
        # ~7us), pairs 3-5 on the scalar ring (free until the first
        # dequant at ~5.5us). No load traffic left in the saturated
        # store window.
        y_sb = yin_pool.tile([128, NPAIRS, S], i8)
        x2s = []

        def load_pair(c, eng):
            # [128(p), 2(bt), 8(r), 64(d)], 512B contiguous per (p, bt)
            x2 = xin_pool.tile([128, 2, 8, 64], i8, tag="x2")
            eng.dma_start(out=x2[:], in_=xv[:, c])
            eng.dma_start(out=y_sb[:, c, :], in_=yv[:, c, :])
            x2s.append(x2)

        for c in range(NPAIRS):
            load_pair(c, nc.sync if c < 3 else nc.scalar)

        # Prep (dequant + transpose) is software-pipelined two pairs ahead
        # of the matmul/store phase: otherwise pair c+1's dequant queues on
        # ACT behind all eight of pair c's PSUM copies and the store stream
        # starves for ~6us at each early pair boundary.
        preps = {}

        def prep(c):
            x2 = x2s[c]
            # dequant x pair -> [128(p), 8(r), 128(bt*64+d)] bf16
            # (permuted at dequant so each transpose input x2bf[:, r, :] is
            #  contiguous: matmul operands allow only one free dimension)
            # Dequants live on the otherwise-idle GPSIMD: with fp16 stores
            # the ACT/DVE pair is the production bottleneck, so every
            # non-PSUM op moves off them.
            x2bf = xbf_pool.tile([128, 8, 128], bf16, tag="x2bf")
            for bt in range(2):
                nc.gpsimd.tensor_scalar_add(
                    x2bf[:, :, bt * 64 : (bt + 1) * 64], x2[:, bt], -az
                )
            # dequant y pair: [128(bt*64+d), 1024(s)] bf16
            y2bf = ybf_pool.tile([128, S], bf16, tag="y2bf")
            nc.gpsimd.tensor_scalar_add(y2bf[:], y_sb[:, c, :], -bz)
            # transpose x: all 8 r-tiles pack into ONE psum bank so the
            # copy-back is a single 1024-elem DVE op instead of 8x128.
            xt = xt_pool.tile([128, 8, 128], bf16, tag="xt")
            tp = tpsum_pool.tile([128, 8, 128], bf16)
            for r in range(8):
                nc.tensor.transpose(tp[:, r, :], x2bf[:, r, :], ident[:])
            nc.vector.tensor_copy(out=xt[:], in_=tp[:])
            preps[c] = (xt, y2bf)

        prep(0)
        prep(1)

        for c in range(NPAIRS):
            xt, y2bf = preps.pop(c)
            # ---- matmuls + scaled PSUM->SBUF copies + stores ----
            # e (bt=0, PE rows 0-63) and o (bt=1, rows 64-127) matmuls are
            # issued adjacently so the row-tiled PE runs them concurrently.
            gsize = 2 if c == 0 else 4  # r-tiles per store
            for g in range(8 // gsize):
                stages = []
                for bt in range(2):
                    stg = stage_pool.tile([128, gsize, S], f16, tag="stage")
                    stages.append(stg)
                for j in range(gsize):
                    m = g * gsize + j
                    pss = []
                    for bt in range(2):
                        ps = mpsum_pool.tile([128, S], f32, tag="mpsum")
                        pss.append(ps)
                    for nh in range(2):
                        for bt in range(2):
                            nc.tensor.matmul(
                                pss[bt][:, nh * 512 : (nh + 1) * 512],
                                xt[bt * 64 : (bt + 1) * 64, m, :],
                                y2bf[bt * 64 : (bt + 1) * 64, nh * 512 : (nh + 1) * 512],
                                start=True,
                                stop=True,
                                tile_position=(bt * 64, 0),
                            )
                    # pair 0: crosswise engine split so each stage fills via
                    # ACT and DVE in parallel (first stores ~1us earlier);
                    # steady state: ACT 9 / DVE 7 of the 16 copies per pair
                    # (measured 1147ns/copy on ACT vs 1218 on DVE, and DVE
                    # also owns the ~1.2us/pair xt copy-back).
                    for bt in range(2):
                        idx = g * 2 * gsize + 2 * j + bt
                        on_act = (
                            idx in (0, 2, 4, 6, 8, 10, 12, 14, 15)
                            if c
                            else ((j + bt) % 2 == 0)
                        )
                        if on_act:
                            nc.scalar.activation(
                                out=stages[bt][:, j, :],
                                in_=pss[bt][:],
                                func=AF.Copy,
                                scale=al,
                            )
                        else:
                            nc.vector.tensor_scalar_mul(
                                stages[bt][:, j, :], pss[bt][:], al
                            )
                for bt in range(2):
                    nc.sync.dma_start(
                        out=ovn[2 * c + bt][:, g * gsize : (g + 1) * gsize, :],
                        in_=stages[bt][:],
                    )
            if c + 2 < NPAIRS:
                prep(c + 2)

    nc.compile()
    _cache[key] = nc
    return nc


def run_sharded(x, y, az, bz, al, trace=False, tmpdir=None):
    """Shard inputs over 8 cores, run, gather. Returns (out, BassKernelResults)."""
    from concourse.bass_utils import run_bass_kernel_spmd

    nc = _build(az, bz, al)
    in_maps = [
        {
            "x": x[i * BPC : (i + 1) * BPC],
            "y": y[i * BPC : (i + 1) * BPC],
        }
        for i in range(N_CORES)
    ]
    res = run_bass_kernel_spmd(
        nc, in_maps, list(range(N_CORES)), trace=trace, tmpdir=tmpdir
    )
    out = np.concatenate(
        [r["out"].astype(np.float32) for r in res.results], axis=0
    )
    return out, res


def kernel(x, y, a_zp, b_zp, alpha):
    x = np.ascontiguousarray(np.asarray(x).astype(np.int8, copy=False))
    y = np.ascontiguousarray(np.asarray(y).astype(np.int8, copy=False))
    az = float(np.asarray(a_zp))
    bz = float(np.asarray(b_zp))
    al = float(np.asarray(alpha))
    out, _ = run_sharded(x, y, az, bz, al)
    return out



# revision 23
# speedup vs baseline: 1.9733x; 1.9733x over previous
"""Trainium2 Bass kernel: batched int8 dequant-BMM.

out[b] = (x[b].f32 - a_zp) @ (y[b].f32 - b_zp) * alpha
  x: [96, 1024, 64] int8, y: [96, 64, 1024] int8 -> out: [96, 1024, 1024] f32

Sharding: batch dim 96 -> 12 per core across 8 cores (pure data parallel).

Per-core pipeline (batch pair at a time; even batch on partitions 0-63,
odd batch on partitions 64-127 so the K=64 contractions row-tile the PE):
  DMA x,y int8 -> ACT dequant to bf16 (exact: integers < 256)
  -> PE transpose x tiles (bf16 identity matmul) -> DVE copy to SBUF
  -> PE matmul bf16 x bf16 -> fp32 PSUM (exact: sums < 2^24)
  -> ACT/DVE copy PSUM->SBUF fused with *alpha, cast to fp16
  -> DMA out fp16 -> host upcasts to fp32.

The PSUM value is the exact integer result; alpha-scale + fp16 round
gives max rel err 2^-11 ~= 4.9e-4 (all nonzero outputs are fp16
normals), 40x inside the 2e-2 gate, and halves the store traffic that
bounds this kernel (memory regime: 4.2 MB output per batch).
"""

import numpy as np

B, S, D = 96, 1024, 64
N_CORES = 8
BPC = B // N_CORES  # batches per core = 12
NPAIRS = BPC // 2

_cache = {}


def _build(az: float, bz: float, al: float):
    key = (az, bz, al)
    if key in _cache:
        return _cache[key]

    from contextlib import ExitStack

    import concourse.mybir as mybir
    import concourse.tile as tile
    from concourse import bacc

    f32 = mybir.dt.float32
    f16 = mybir.dt.float16
    bf16 = mybir.dt.bfloat16
    i8 = mybir.dt.int8
    AF = mybir.ActivationFunctionType

    nc = bacc.Bacc(
        "TRN2", target_bir_lowering=False, debug=False, num_devices=N_CORES
    )
    x_d = nc.dram_tensor("x", [BPC, S, D], i8, kind="ExternalInput").ap()
    y_d = nc.dram_tensor("y", [BPC, D, S], i8, kind="ExternalInput").ap()
    o_d = nc.dram_tensor("out", [BPC, S, S], f16, kind="ExternalOutput").ap()

    # Row-residue tiling: m-tile r (r in 0..7) covers rows {8p + r}.
    # This makes the x load contiguous per partition (512B runs) and the
    # store rows of one partition contiguous in DRAM (gsize*4KB runs).
    # x[2c+bt, 8p+r, d] -> xv[p, c, bt, r, d]
    xv = x_d.rearrange("(c b2) (p r) d -> p c b2 r d", b2=2, p=128)
    # y[2c+bt, d, s] -> yv[bt*64+d, c, s]  (contiguous in DRAM)
    yv = y_d.rearrange("(c b2) d s -> (b2 d) c s", b2=2)
    # out[b, 8p+r, t] <- ovn[b, p, r, t]
    ovn = o_d.rearrange("b (p r) t -> b p r t", p=128, r=8)

    with tile.TileContext(nc) as tc, ExitStack() as ctx:
        const_pool = ctx.enter_context(tc.tile_pool(name="const", bufs=1))
        # all 6 x-pair tiles live at once: loads are issued up front
        xin_pool = ctx.enter_context(tc.tile_pool(name="xin", bufs=NPAIRS))
        yin_pool = ctx.enter_context(tc.tile_pool(name="yin", bufs=1))
        xbf_pool = ctx.enter_context(tc.tile_pool(name="xbf", bufs=2))
        ybf_pool = ctx.enter_context(tc.tile_pool(name="ybf", bufs=3))
        xt_pool = ctx.enter_context(tc.tile_pool(name="xt", bufs=3))
        stage_pool = ctx.enter_context(tc.tile_pool(name="stage", bufs=12))
        tpsum_pool = ctx.enter_context(
            tc.tile_pool(name="tpsum", bufs=2, space="PSUM")
        )
        mpsum_pool = ctx.enter_context(
            tc.tile_pool(name="mpsum", bufs=3, space="PSUM")
        )

        # Identity as a baked constant (avoids serializing gpsimd early).
        import ml_dtypes

        ident_dram = nc.inline_tensor(
            np.eye(128, dtype=ml_dtypes.bfloat16), name="ident128"
        ).ap()
        # Constants ride the scalar ring (whose first real cargo, pairs
        # 3-5, isn't needed until ~40us) so pair-0 x/y lead the sync ring
        # and the dequant->transpose->matmul chain starts ~2us earlier.
        ident = const_pool.tile([128, 128], bf16)
        nc.scalar.dma_start(out=ident[:], in_=ident_dram)

        # HAM warmup: ~3.4us of dummy matmuls while PE is otherwise idle
        # (waiting on loads) flips the PE clock gate from 1.2 to 2.4 GHz
        # before the real matmul stream starts at ~7.5us. Result is never
        # read. Data arrives via sync DMA (~3.5us) — engine memsets can't
        # run before ~6us (preamble), which is too late to matter.
        warm_dram = nc.inline_tensor(
            np.ones((128, 512), dtype=ml_dtypes.bfloat16), name="warm512"
        ).ap()
        warm_sb = const_pool.tile([128, 512], bf16)
        nc.scalar.dma_start(out=warm_sb[:], in_=warm_dram)
        warm_ps = mpsum_pool.tile([128, S], f32, tag="mpsum")
        for w in range(8):
            nh = w % 2
            nc.tensor.matmul(
                warm_ps[:, nh * 512 : (nh + 1) * 512],
                warm_sb[:, :128],
                warm_sb[:],
                start=True,
                stop=True,
            )

        # All loads ride HWDGE (no SWDGE at all: SDMA engines 7/15 are
        # documented slower under SWDGE descriptor-ring port traffic and
        # intermittently straggle the store stream by ~17us). Everything
        # loads up front into the otherwise-idle 0-10us DMA window:
        # pairs 0-2 on the sync ring (free until the first store at
        # ~7us), pairs 3-5 on the scalar ring (free until the first
        # dequant at ~5.5us). No load traffic left in the saturated
        # store window.
        y_sb = yin_pool.tile([128, NPAIRS, S], i8)
        x2s = []

        def load_pair(c, eng):
            # [128(p), 2(bt), 8(r), 64(d)], 512B contiguous per (p, bt)
            x2 = xin_pool.tile([128, 2, 8, 64], i8, tag="x2")
            eng.dma_start(out=x2[:], in_=xv[:, c])
            eng.dma_start(out=y_sb[:, c, :], in_=yv[:, c, :])
            x2s.append(x2)

        for c in range(NPAIRS):
            load_pair(c, nc.sync if c < 3 else nc.scalar)

        # Prep (dequant + transpose) is software-pipelined two pairs ahead
        # of the matmul/store phase: otherwise pair c+1's dequant queues on
        # ACT behind all eight of pair c's PSUM copies and the store stream
        # starves for ~6us at each early pair boundary.
        preps = {}

        def prep(c):
            x2 = x2s[c]
            # dequant x pair -> [128(p), 8(r), 128(bt*64+d)] bf16
            # (permuted at dequant so each transpose input x2bf[:, r, :] is
            #  contiguous: matmul operands allow only one free dimension)
            # (GPSIMD dequant measured ~9.8us/instr — int8->bf16 falls off
            # the fast path — so dequants stay on ACT.)
            x2bf = xbf_pool.tile([128, 8, 128], bf16, tag="x2bf")
            # single instr via 3-D out AP (bt,r,d): one 1024-elem ACTIVATE
            # (1147ns) instead of two 512s (2x720ns)
            x2bf_v = x2bf.rearrange("p r (b2 d) -> p b2 r d", b2=2)
            nc.scalar.activation(
                out=x2bf_v,
                in_=x2[:],
                func=AF.Copy,
                bias=-az,
                scale=1.0,
            )
            # dequant y pair: [128(bt*64+d), 1024(s)] bf16
            y2bf = ybf_pool.tile([128, S], bf16, tag="y2bf")
            nc.scalar.activation(
                out=y2bf[:], in_=y_sb[:, c, :], func=AF.Copy, bias=-bz, scale=1.0
            )
            # transpose x: all 8 r-tiles pack into ONE psum bank so the
            # copy-back is a single 1024-elem DVE op instead of 8x128.
            xt = xt_pool.tile([128, 8, 128], bf16, tag="xt")
            tp = tpsum_pool.tile([128, 8, 128], bf16)
            for r in range(8):
                nc.tensor.transpose(tp[:, r, :], x2bf[:, r, :], ident[:])
            nc.vector.tensor_copy(out=xt[:], in_=tp[:])
            preps[c] = (xt, y2bf)

        prep(0)
        prep(1)

        for c in range(NPAIRS):
            xt, y2bf = preps.pop(c)
            # ---- matmuls + scaled PSUM->SBUF copies + stores ----
            # e (bt=0, PE rows 0-63) and o (bt=1, rows 64-127) matmuls are
            # issued adjacently so the row-tiled PE runs them concurrently.
            gsize = 2 if c == 0 else 4  # r-tiles per store
            for g in range(8 // gsize):
                stages = []
                for bt in range(2):
                    stg = stage_pool.tile([128, gsize, S], f16, tag="stage")
                    stages.append(stg)
                for j in range(gsize):
                    m = g * gsize + j
                    pss = []
                    for bt in range(2):
                        ps = mpsum_pool.tile([128, S], f32, tag="mpsum")
                        pss.append(ps)
                    for nh in range(2):
                        for bt in range(2):
                            nc.tensor.matmul(
                                pss[bt][:, nh * 512 : (nh + 1) * 512],
                                xt[bt * 64 : (bt + 1) * 64, m, :],
                                y2bf[bt * 64 : (bt + 1) * 64, nh * 512 : (nh + 1) * 512],
                                start=True,
                                stop=True,
                                tile_position=(bt * 64, 0),
                            )
                    # pair 0: crosswise engine split so each stage fills via
                    # ACT and DVE in parallel (first stores ~1us earlier);
                    # steady state: the optimal ACT share of the 16 copies
                    # is fractional (~7.4: ACT owns the ~2.3us dequant at
                    # 1147ns/copy, DVE owns the 0.7us xt at 1218ns/copy),
                    # so alternate 7/8 by pair parity.
                    for bt in range(2):
                        idx = g * 2 * gsize + 2 * j + bt
                        act_set = (
                            (0, 2, 4, 7, 9, 11, 14)
                            if c % 2
                            else (0, 2, 4, 6, 9, 11, 13, 15)
                        )
                        on_act = (
                            idx in act_set if c else ((j + bt) % 2 == 0)
                        )
                        if on_act:
                            nc.scalar.activation(
                                out=stages[bt][:, j, :],
                                in_=pss[bt][:],
                                func=AF.Copy,
                                scale=al,
                            )
                        else:
                            nc.vector.tensor_scalar_mul(
                                stages[bt][:, j, :], pss[bt][:], al
                            )
                for bt in range(2):
                    nc.sync.dma_start(
                        out=ovn[2 * c + bt][:, g * gsize : (g + 1) * gsize, :],
                        in_=stages[bt][:],
                    )
            if c + 2 < NPAIRS:
                prep(c + 2)

    nc.compile()
    _cache[key] = nc
    return nc


def run_sharded(x, y, az, bz, al, trace=False, tmpdir=None):
    """Shard inputs over 8 cores, run, gather. Returns (out, BassKernelResults)."""
    from concourse.bass_utils import run_bass_kernel_spmd

    nc = _build(az, bz, al)
    in_maps = [
        {
            "x": x[i * BPC : (i + 1) * BPC],
            "y": y[i * BPC : (i + 1) * BPC],
        }
        for i in range(N_CORES)
    ]
    res = run_bass_kernel_spmd(
        nc, in_maps, list(range(N_CORES)), trace=trace, tmpdir=tmpdir
    )
    out = np.concatenate(
        [r["out"].astype(np.float32) for r in res.results], axis=0
    )
    return out, res


def kernel(x, y, a_zp, b_zp, alpha):
    x = np.ascontiguousarray(np.asarray(x).astype(np.int8, copy=False))
    y = np.ascontiguousarray(np.asarray(y).astype(np.int8, copy=False))
    az = float(np.asarray(a_zp))
    bz = float(np.asarray(b_zp))
    al = float(np.asarray(alpha))
    out, _ = run_sharded(x, y, az, bz, al)
    return out



# revision 24
# speedup vs baseline: 2.5687x; 1.3017x over previous
"""Trainium2 Bass kernel: batched int8 dequant-BMM.

out[b] = (x[b].f32 - a_zp) @ (y[b].f32 - b_zp) * alpha
  x: [96, 1024, 64] int8, y: [96, 64, 1024] int8 -> out: [96, 1024, 1024] f32

Sharding: batch dim 96 -> 12 per core across 8 cores (pure data parallel).

The store stream saturates HBM (~358 GB/s/core), which trips the chip's
activity throttle: the PE is clock-gated to K=4/8 (~1.2 col/ns) for the
whole main phase, making PE column count the binding roofline. Hence:
  - x is pre-transposed on the HOST (numpy) so the kernel needs no PE
    transposes (saves 6144 PE columns/core + the DVE copy-backs + the
    identity constant; lhsT reads the dequanted x_T with a stride-8
    free-dim AP instead).
  - output is stored as fp16 and upcast on the host: the PSUM value is
    the exact integer result; alpha-scale + fp16 round gives max rel
    err 2^-11 ~= 4.9e-4, 40x inside the 2e-2 gate, and halves the
    store traffic.

Per-core pipeline (batch pair at a time; even batch on PE rows 0-63,
odd batch on rows 64-127 so the K=64 contractions row-tile the PE):
  DMA x_T,y int8 -> ACT dequant to bf16 (exact: integers < 256)
  -> PE matmul bf16 -> fp32 PSUM (exact: sums < 2^24)
  -> ACT/DVE copy PSUM->SBUF fused with *alpha, cast fp16 -> DMA out.

All DMA rides the two HWDGE rings. dma_start costs ~630ns on the
issuing engine, so the sync ring carries everything (loads interleaved
between store groups) and ACT issues none; the scalar ring is unused.
HAM warmup (~3.4us of dummy matmuls on memset data while PE waits on
the first loads) buys K=8/8 for the first ~2 pairs of real matmuls.
"""

import numpy as np

B, S, D = 96, 1024, 64
N_CORES = 8
BPC = B // N_CORES  # batches per core = 12
NPAIRS = BPC // 2

_cache = {}


def _build(az: float, bz: float, al: float):
    key = (az, bz, al)
    if key in _cache:
        return _cache[key]

    from contextlib import ExitStack

    import concourse.mybir as mybir
    import concourse.tile as tile
    from concourse import bacc

    f32 = mybir.dt.float32
    f16 = mybir.dt.float16
    bf16 = mybir.dt.bfloat16
    i8 = mybir.dt.int8
    AF = mybir.ActivationFunctionType

    nc = bacc.Bacc(
        "TRN2", target_bir_lowering=False, debug=False, num_devices=N_CORES
    )
    x_d = nc.dram_tensor("xt", [BPC, D, S], i8, kind="ExternalInput").ap()
    y_d = nc.dram_tensor("y", [BPC, D, S], i8, kind="ExternalInput").ap()
    o_d = nc.dram_tensor("out", [BPC, S, S], f16, kind="ExternalOutput").ap()

    # partition = bt*64+d; per (partition, pair) a contiguous 1KB DRAM run
    xv = x_d.rearrange("(c b2) d s -> (b2 d) c s", b2=2)
    yv = y_d.rearrange("(c b2) d s -> (b2 d) c s", b2=2)
    # out[b, 8p+r, t] <- ovn[b, p, r, t]: m-tile r covers rows {8p + r},
    # so one store's rows per partition are gsize*2KB contiguous in DRAM
    ovn = o_d.rearrange("b (p r) t -> b p r t", p=128, r=8)

    with tile.TileContext(nc) as tc, ExitStack() as ctx:
        const_pool = ctx.enter_context(tc.tile_pool(name="const", bufs=1))
        xin_pool = ctx.enter_context(tc.tile_pool(name="xin", bufs=1))
        yin_pool = ctx.enter_context(tc.tile_pool(name="yin", bufs=1))
        xbf_pool = ctx.enter_context(tc.tile_pool(name="xbf", bufs=3))
        ybf_pool = ctx.enter_context(tc.tile_pool(name="ybf", bufs=3))
        stage_pool = ctx.enter_context(tc.tile_pool(name="stage", bufs=12))
        mpsum_pool = ctx.enter_context(
            tc.tile_pool(name="mpsum", bufs=4, space="PSUM")
        )

        # HAM warmup: dummy matmuls while PE is otherwise idle (waiting
        # on loads) hold the clock gate at K=8/8 into the first real
        # pairs. Data comes from a DVE memset (no DMA, ready ~4us).
        warm_sb = const_pool.tile([128, 512], bf16)
        nc.vector.memset(warm_sb[:], 1.0)
        warm_ps = mpsum_pool.tile([128, S], f32, tag="mpsum")
        for w in range(8):
            nh = w % 2
            nc.tensor.matmul(
                warm_ps[:, nh * 512 : (nh + 1) * 512],
                warm_sb[:, :128],
                warm_sb[:],
                start=True,
                stop=True,
            )

        x_sb = xin_pool.tile([128, NPAIRS, S], i8)
        y_sb = yin_pool.tile([128, NPAIRS, S], i8)

        def load_pair(c):
            nc.sync.dma_start(out=x_sb[:, c, :], in_=xv[:, c, :])
            nc.sync.dma_start(out=y_sb[:, c, :], in_=yv[:, c, :])

        load_pair(0)
        load_pair(1)

        # Prep (dequant) is software-pipelined two pairs ahead of the
        # matmul/store phase so pair boundaries don't stall the copies.
        preps = {}

        def prep(c):
            xbf = xbf_pool.tile([128, S], bf16, tag="xbf")
            nc.scalar.activation(
                out=xbf[:], in_=x_sb[:, c, :], func=AF.Copy, bias=-az, scale=1.0
            )
            ybf = ybf_pool.tile([128, S], bf16, tag="ybf")
            nc.scalar.activation(
                out=ybf[:], in_=y_sb[:, c, :], func=AF.Copy, bias=-bz, scale=1.0
            )
            # lhsT view: [128(bt,d), 8(r), 128(p)] with free stride 8
            preps[c] = (xbf.rearrange("q (p r) -> q r p", r=8), ybf)

        prep(0)
        prep(1)

        for c in range(NPAIRS):
            xtv, ybf = preps.pop(c)
            # gsize = r-tiles per store; small first groups for an early
            # first store, small last groups to shorten the drain tail
            gplan = (
                [2, 2, 4] if c == 0
                else ([4, 2, 2] if c == NPAIRS - 1 else [4, 4])
            )
            g0 = 0
            for gi, gsize in enumerate(gplan):
                stages = []
                for bt in range(2):
                    stg = stage_pool.tile([128, gsize, S], f16, tag="stage")
                    stages.append(stg)
                for j in range(gsize):
                    m = g0 + j
                    pss = []
                    for bt in range(2):
                        ps = mpsum_pool.tile([128, S], f32, tag="mpsum")
                        pss.append(ps)
                    # e/o matmuls issued adjacently so the row-tiled PE
                    # overlaps their drain/fill
                    for nh in range(2):
                        for bt in range(2):
                            nc.tensor.matmul(
                                pss[bt][:, nh * 512 : (nh + 1) * 512],
                                xtv[bt * 64 : (bt + 1) * 64, m, :],
                                ybf[bt * 64 : (bt + 1) * 64, nh * 512 : (nh + 1) * 512],
                                start=True,
                                stop=True,
                                tile_position=(bt * 64, 0),
                            )
                    # pair 0: crosswise split so each stage fills via ACT
                    # and DVE in parallel (first stores ~1us earlier);
                    # steady state: the optimal ACT share of the 16
                    # copies/pair is fractional (~7.4: ACT owns the
                    # ~2.3us dequant at 1147ns/copy vs DVE's 1218ns), so
                    # alternate 7/8 by pair parity.
                    for bt in range(2):
                        idx = (g0 + j) * 2 + bt
                        act_set = (
                            (0, 2, 4, 7, 9, 11, 14)
                            if c % 2
                            else (0, 2, 4, 6, 9, 11, 13, 15)
                        )
                        on_act = (
                            idx in act_set if c else ((j + bt) % 2 == 0)
                        )
                        if on_act:
                            nc.scalar.activation(
                                out=stages[bt][:, j, :],
                                in_=pss[bt][:],
                                func=AF.Copy,
                                scale=al,
                            )
                        else:
                            nc.vector.tensor_scalar_mul(
                                stages[bt][:, j, :], pss[bt][:], al
                            )
                for bt in range(2):
                    nc.sync.dma_start(
                        out=ovn[2 * c + bt][:, g0 : g0 + gsize, :],
                        in_=stages[bt][:],
                    )
                # interleave the next loads behind the first store issues
                # (sync-ring issue costs ~630ns each; ACT issues nothing)
                if gi == 0 and c + 2 < NPAIRS:
                    load_pair(c + 2)
                g0 += gsize
            if c + 2 < NPAIRS:
                prep(c + 2)

    nc.compile()
    _cache[key] = nc
    return nc


def run_sharded(x, y, az, bz, al, trace=False, tmpdir=None):
    """Shard inputs over 8 cores, run, gather. Returns (out, BassKernelResults)."""
    from concourse.bass_utils import run_bass_kernel_spmd

    nc = _build(az, bz, al)
    xt = np.ascontiguousarray(x.transpose(0, 2, 1))  # host pre-transpose
    in_maps = [
        {
            "xt": xt[i * BPC : (i + 1) * BPC],
            "y": y[i * BPC : (i + 1) * BPC],
        }
        for i in range(N_CORES)
    ]
    res = run_bass_kernel_spmd(
        nc, in_maps, list(range(N_CORES)), trace=trace, tmpdir=tmpdir
    )
    out = np.concatenate(
        [r["out"].astype(np.float32) for r in res.results], axis=0
    )
    return out, res


def kernel(x, y, a_zp, b_zp, alpha):
    x = np.ascontiguousarray(np.asarray(x).astype(np.int8, copy=False))
    y = np.ascontiguousarray(np.asarray(y).astype(np.int8, copy=False))
    az = float(np.asarray(a_zp))
    bz = float(np.asarray(b_zp))
    al = float(np.asarray(alpha))
    out, _ = run_sharded(x, y, az, bz, al)
    return out


# revision 26
# speedup vs baseline: 2.5744x; 1.0022x over previous
"""Trainium2 Bass kernel: batched int8 dequant-BMM.

out[b] = (x[b].f32 - a_zp) @ (y[b].f32 - b_zp) * alpha
  x: [96, 1024, 64] int8, y: [96, 64, 1024] int8 -> out: [96, 1024, 1024] f32

Sharding: batch dim 96 -> 12 per core across 8 cores (pure data parallel).

The store stream saturates HBM (~358 GB/s/core), which trips the chip's
activity throttle: the PE is clock-gated to K=4/8 (~1.2 col/ns) for the
whole main phase, making PE column count the binding roofline. Hence:
  - x is pre-transposed on the HOST (numpy) so the kernel needs no PE
    transposes (saves 6144 PE columns/core + the DVE copy-backs + the
    identity constant; lhsT reads the dequanted x_T with a stride-8
    free-dim AP instead).
  - output is stored as fp16 and upcast on the host: the PSUM value is
    the exact integer result; alpha-scale + fp16 round gives max rel
    err 2^-11 ~= 4.9e-4, 40x inside the 2e-2 gate, and halves the
    store traffic.

Per-core pipeline (batch pair at a time; even batch on PE rows 0-63,
odd batch on rows 64-127 so the K=64 contractions row-tile the PE):
  DMA x_T,y int8 -> ACT dequant to bf16 (exact: integers < 256)
  -> PE matmul bf16 -> fp32 PSUM (exact: sums < 2^24)
  -> ACT/DVE copy PSUM->SBUF fused with *alpha, cast fp16 -> DMA out.

All DMA rides the two HWDGE rings. dma_start costs ~630ns on the
issuing engine, so the sync ring carries everything (loads interleaved
between store groups) and ACT issues none; the scalar ring is unused.
HAM warmup (~3.4us of dummy matmuls on memset data while PE waits on
the first loads) buys K=8/8 for the first ~2 pairs of real matmuls.
"""

import numpy as np

B, S, D = 96, 1024, 64
N_CORES = 8
BPC = B // N_CORES  # batches per core = 12
NPAIRS = BPC // 2

_cache = {}


def _build(az: float, bz: float, al: float):
    key = (az, bz, al)
    if key in _cache:
        return _cache[key]

    from contextlib import ExitStack

    import concourse.mybir as mybir
    import concourse.tile as tile
    from concourse import bacc

    f32 = mybir.dt.float32
    f16 = mybir.dt.float16
    bf16 = mybir.dt.bfloat16
    i8 = mybir.dt.int8
    AF = mybir.ActivationFunctionType

    nc = bacc.Bacc(
        "TRN2", target_bir_lowering=False, debug=False, num_devices=N_CORES
    )
    x_d = nc.dram_tensor("xt", [BPC, D, S], i8, kind="ExternalInput").ap()
    y_d = nc.dram_tensor("y", [BPC, D, S], i8, kind="ExternalInput").ap()
    o_d = nc.dram_tensor("out", [BPC, S, S], f16, kind="ExternalOutput").ap()

    # partition = bt*64+d; per (partition, pair) a contiguous 1KB DRAM run
    xv = x_d.rearrange("(c b2) d s -> (b2 d) c s", b2=2)
    yv = y_d.rearrange("(c b2) d s -> (b2 d) c s", b2=2)
    # out[b, 8p+r, t] <- ovn[b, p, r, t]: m-tile r covers rows {8p + r},
    # so one store's rows per partition are gsize*2KB contiguous in DRAM
    ovn = o_d.rearrange("b (p r) t -> b p r t", p=128, r=8)

    with tile.TileContext(nc) as tc, ExitStack() as ctx:
        const_pool = ctx.enter_context(tc.tile_pool(name="const", bufs=1))
        xin_pool = ctx.enter_context(tc.tile_pool(name="xin", bufs=1))
        yin_pool = ctx.enter_context(tc.tile_pool(name="yin", bufs=1))
        xbf_pool = ctx.enter_context(tc.tile_pool(name="xbf", bufs=3))
        ybf_pool = ctx.enter_context(tc.tile_pool(name="ybf", bufs=3))
        stage_pool = ctx.enter_context(tc.tile_pool(name="stage", bufs=12))
        mpsum_pool = ctx.enter_context(
            tc.tile_pool(name="mpsum", bufs=4, space="PSUM")
        )

        # HAM warmup: dummy matmuls while PE is otherwise idle (waiting
        # on loads) hold the clock gate at K=8/8 into the first real
        # pairs. Data comes from a DVE memset (no DMA, ready ~4us).
        warm_sb = const_pool.tile([128, 512], bf16)
        nc.vector.memset(warm_sb[:], 1.0)
        warm_ps = mpsum_pool.tile([128, S], f32, tag="mpsum")
        for w in range(8):
            nh = w % 2
            nc.tensor.matmul(
                warm_ps[:, nh * 512 : (nh + 1) * 512],
                warm_sb[:, :128],
                warm_sb[:],
                start=True,
                stop=True,
            )

        x_sb = xin_pool.tile([128, NPAIRS, S], i8)
        y_sb = yin_pool.tile([128, NPAIRS, S], i8)

        def load_pair(c):
            nc.sync.dma_start(out=x_sb[:, c, :], in_=xv[:, c, :])
            nc.sync.dma_start(out=y_sb[:, c, :], in_=yv[:, c, :])

        # All loads issue up front: the whole 1.57MB flows during the
        # pre-store ramp while HBM is otherwise idle, instead of stealing
        # ~4us of store bandwidth mid-stream.
        for c in range(NPAIRS):
            load_pair(c)

        # Prep (dequant) is software-pipelined two pairs ahead of the
        # matmul/store phase so pair boundaries don't stall the copies.
        preps = {}

        def prep(c):
            xbf = xbf_pool.tile([128, S], bf16, tag="xbf")
            nc.scalar.activation(
                out=xbf[:], in_=x_sb[:, c, :], func=AF.Copy, bias=-az, scale=1.0
            )
            ybf = ybf_pool.tile([128, S], bf16, tag="ybf")
            nc.scalar.activation(
                out=ybf[:], in_=y_sb[:, c, :], func=AF.Copy, bias=-bz, scale=1.0
            )
            # lhsT view: [128(bt,d), 8(r), 128(p)] with free stride 8
            preps[c] = (xbf.rearrange("q (p r) -> q r p", r=8), ybf)

        prep(0)
        prep(1)

        for c in range(NPAIRS):
            xtv, ybf = preps.pop(c)
            # gsize = r-tiles per store; small first groups for an early
            # first store, small last groups to shorten the drain tail
            gplan = (
                [2, 2, 4] if c == 0
                else ([4, 2, 2] if c == NPAIRS - 1 else [4, 4])
            )
            g0 = 0
            for gi, gsize in enumerate(gplan):
                stages = []
                for bt in range(2):
                    stg = stage_pool.tile([128, gsize, S], f16, tag="stage")
                    stages.append(stg)
                for j in range(gsize):
                    m = g0 + j
                    pss = []
                    for bt in range(2):
                        ps = mpsum_pool.tile([128, S], f32, tag="mpsum")
                        pss.append(ps)
                    # e/o matmuls issued adjacently so the row-tiled PE
                    # overlaps their drain/fill
                    for nh in range(2):
                        for bt in range(2):
                            nc.tensor.matmul(
                                pss[bt][:, nh * 512 : (nh + 1) * 512],
                                xtv[bt * 64 : (bt + 1) * 64, m, :],
                                ybf[bt * 64 : (bt + 1) * 64, nh * 512 : (nh + 1) * 512],
                                start=True,
                                stop=True,
                                tile_position=(bt * 64, 0),
                            )
                    # pair 0: crosswise split so each stage fills via ACT
                    # and DVE in parallel (first stores ~1us earlier);
                    # steady state: the optimal ACT share of the 16
                    # copies/pair is fractional (~7.4: ACT owns the
                    # ~2.3us dequant at 1147ns/copy vs DVE's 1218ns), so
                    # alternate 7/8 by pair parity.
                    for bt in range(2):
                        idx = (g0 + j) * 2 + bt
                        act_set = (
                            (0, 2, 4, 7, 9, 11, 14)
                            if c % 2
                            else (0, 2, 4, 6, 9, 11, 13, 15)
                        )
                        on_act = (
                            idx in act_set if c else ((j + bt) % 2 == 0)
                        )
                        if on_act:
                            nc.scalar.activation(
                                out=stages[bt][:, j, :],
                                in_=pss[bt][:],
                                func=AF.Copy,
                                scale=al,
                            )
                        else:
                            nc.vector.tensor_scalar_mul(
                                stages[bt][:, j, :], pss[bt][:], al
                            )
                for bt in range(2):
                    nc.sync.dma_start(
                        out=ovn[2 * c + bt][:, g0 : g0 + gsize, :],
                        in_=stages[bt][:],
                    )
                g0 += gsize
            if c + 2 < NPAIRS:
                prep(c + 2)

    nc.compile()
    _cache[key] = nc
    return nc


def run_sharded(x, y, az, bz, al, trace=False, tmpdir=None):
    """Shard inputs over 8 cores, run, gather. Returns (out, BassKernelResults)."""
    from concourse.bass_utils import run_bass_kernel_spmd

    nc = _build(az, bz, al)
    xt = np.ascontiguousarray(x.transpose(0, 2, 1))  # host pre-transpose
    in_maps = [
        {
            "xt": xt[i * BPC : (i + 1) * BPC],
            "y": y[i * BPC : (i + 1) * BPC],
        }
        for i in range(N_CORES)
    ]
    res = run_bass_kernel_spmd(
        nc, in_maps, list(range(N_CORES)), trace=trace, tmpdir=tmpdir
    )
    out = np.concatenate(
        [r["out"].astype(np.float32) for r in res.results], axis=0
    )
    return out, res


def kernel(x, y, a_zp, b_zp, alpha):
    x = np.ascontiguousarray(np.asarray(x).astype(np.int8, copy=False))
    y = np.ascontiguousarray(np.asarray(y).astype(np.int8, copy=False))
    az = float(np.asarray(a_zp))
    bz = float(np.asarray(b_zp))
    al = float(np.asarray(alpha))
    out, _ = run_sharded(x, y, az, bz, al)
    return out
